# revision 17
# baseline (speedup 1.0000x reference)
"""Anomaly MultiHeadAttention Trainium2 kernel.

Data-parallel over batch B=8 across 8 NeuronCores (one batch element per core,
no collectives). Per core:

  - qT/kT projections produce transposed layouts [dm, l] feeding the scores
    matmul directly; v is produced in natural [s, dm] layout with a ones
    column appended per head so the attention-V matmul also yields softmax
    row sums for free.
  - scoresT[s, l] per head via PE (bf16, fp32 PSUM accumulate); softmax exp on
    ScalarE directly from PSUM with the 1/sqrt(E) scale folded into the
    activation's free affine.  No max subtraction (scores are ~N(0,1)).
  - AV matmul gives out_VT[e,l] + sums row; normalization divides via a
    PE-broadcast reciprocal; result is exactly the lhsT of the final Wo
    projection.
  - prior = 1/(sqrt(2pi)*sigma) * exp(-d^2/(2 sigma^2)) underflows to exactly
    0 in fp32 for |l-s| >= 29 (sigma <= 2.0002), so only a 256-wide band is
    computed (one ScalarE exp per tile: exp(scale_l * d2 + bias_l) with
    per-partition scale/bias precomputed on host) and the host scatters the
    band into the zero-filled full [B,H,L,L] output.

Self-contained: hardcodes shapes; host-side prep = transposes, bf16 casts and
the tiny sigma->scale/bias precompute ([B,L,H], 0.2% of total FLOPs).
"""

import os
from contextlib import ExitStack

import ml_dtypes
import numpy as np

import concourse.bass as bass
import concourse.tile as tile
from concourse import bacc, mybir
from concourse.bass import ts
from concourse.bass_utils import run_bass_kernel_spmd

B, L, DM, H = 8, 1024, 512, 8
E = DM // H  # 64
BAND = 256  # prior band width (covers |l-s| <= 64 minimum for every row)
# window start per 128-row l-chunk (clamped so every chunk reads 256 cols)
W0 = [0, 64, 192, 320, 448, 576, 704, 768]

F32 = mybir.dt.float32
BF16 = mybir.dt.bfloat16

LAST_RESULT = None  # BassKernelResults of the most recent run (for test.py)


def _build():
    nc = bacc.Bacc(
        "TRN2",
        target_bir_lowering=False,
        debug=False,
        enable_asserts=False,
        num_devices=8,
    )

    xqT = nc.dram_tensor("xqT", [DM, L], BF16, kind="ExternalInput").ap()
    xkT = nc.dram_tensor("xkT", [DM, L], BF16, kind="ExternalInput").ap()
    xvT = nc.dram_tensor("xvT", [DM, L], BF16, kind="ExternalInput").ap()
    wq = nc.dram_tensor("wq", [DM, DM], BF16, kind="ExternalInput").ap()
    wk = nc.dram_tensor("wk", [DM, DM], BF16, kind="ExternalInput").ap()
    wv = nc.dram_tensor("wv", [DM, DM], BF16, kind="ExternalInput").ap()
    wo = nc.dram_tensor("wo", [DM, DM], BF16, kind="ExternalInput").ap()
    d2t = nc.dram_tensor("d2t", [128, 384], F32, kind="ExternalInput").ap()
    pscale = nc.dram_tensor("pscale", [L, H], F32, kind="ExternalInput").ap()
    pbias = nc.dram_tensor("pbias", [L, H], F32, kind="ExternalInput").ap()
    sel = nc.dram_tensor("sel", [128, 256], F32, kind="ExternalInput").ap()

    outp = nc.dram_tensor("outp", [L, DM], F32, kind="ExternalOutput").ap()
    priorband = nc.dram_tensor("priorband", [H, L, BAND], F32, kind="ExternalOutput").ap()

    EXP = mybir.ActivationFunctionType.Exp
    MULT = mybir.AluOpType.mult
    ADD = mybir.AluOpType.add

    with tile.TileContext(nc) as tc, ExitStack() as ctx:
        const = ctx.enter_context(tc.tile_pool(name="const", bufs=1))
        priorp = ctx.enter_context(tc.tile_pool(name="priorp", bufs=2))
        expp = ctx.enter_context(tc.tile_pool(name="expp", bufs=3))
        smallp = ctx.enter_context(tc.tile_pool(name="smallp", bufs=3))
        avp = ctx.enter_context(tc.tile_pool(name="avp", bufs=6))
        outsb = ctx.enter_context(tc.tile_pool(name="outsb", bufs=2))
        ps_scores = ctx.enter_context(tc.tile_pool(name="ps_scores", bufs=2, space="PSUM"))
        ps_av = ctx.enter_context(tc.tile_pool(name="ps_av", bufs=2, space="PSUM"))
        ps_proj = ctx.enter_context(tc.tile_pool(name="ps_proj", bufs=2, space="PSUM"))

        # ---- ALL input loads first: the Sync engine issues DMAs in program
        # order, so every load must precede any compute-dependent store (a
        # blocked store would head-of-line-block the loads).
        d2_sb = const.tile([128, 384], F32)
        nc.sync.dma_start(d2_sb[:], d2t)
        psc_sb = const.tile([128, 8, H], F32)
        nc.sync.dma_start(psc_sb[:], pscale.rearrange("(c p) h -> p c h", p=128))
        pbi_sb = const.tile([128, 8, H], F32)
        nc.sync.dma_start(pbi_sb[:], pbias.rearrange("(c p) h -> p c h", p=128))
        sel_sb = const.tile([128, 256], F32)
        nc.sync.dma_start(sel_sb[:], sel)

        wq_sb = const.tile([128, 4, DM], BF16)
        wk_sb = const.tile([128, 4, DM], BF16)
        wv_sb = const.tile([128, 4, DM], BF16)
        wo_sb = const.tile([128, 4, DM], BF16)
        xq_sb = const.tile([128, 4, L], BF16)
        xk_sb = const.tile([128, 4, L], BF16)
        xv_sb = const.tile([128, 4, L], BF16)
        for kc in range(4):
            nc.sync.dma_start(wq_sb[:, kc, :], wq.rearrange("(c p) n -> p c n", p=128)[:, kc, :])
            nc.sync.dma_start(xq_sb[:, kc, :], xqT.rearrange("(c p) l -> p c l", p=128)[:, kc, :])
            nc.sync.dma_start(wk_sb[:, kc, :], wk.rearrange("(c p) n -> p c n", p=128)[:, kc, :])
            nc.sync.dma_start(xk_sb[:, kc, :], xkT.rearrange("(c p) l -> p c l", p=128)[:, kc, :])
            nc.sync.dma_start(wv_sb[:, kc, :], wv.rearrange("(c p) n -> p c n", p=128)[:, kc, :])
            nc.sync.dma_start(xv_sb[:, kc, :], xvT.rearrange("(c p) l -> p c l", p=128)[:, kc, :])
        nc.sync.dma_start(wo_sb[:], wo.rearrange("(c p) n -> p c n", p=128))

        # softmax sums staging: rows live at partitions 0/32/64/96 (32-aligned)
        sums_all = const.tile([128, 512], F32)
        nc.vector.memset(sums_all[:], 1.0)

        # ---- prior band: GPSIMD prescale + one big ACT exp per l_chunk ------
        # prior tile = exp(scale_l * d2 + bias_l); all 8 heads batched in the
        # free dim so ScalarE pays the per-op overhead once per l_chunk.
        def emit_prior(lc):
            shift = 128 * lc - W0[lc]  # 0, 64 or 128
            d2s = d2_sb[:, 128 - shift : 384 - shift]
            pre = priorp.tile([128, H, BAND], F32, tag="pre")
            for h in range(H):
                nc.gpsimd.tensor_scalar(
                    pre[:, h, :],
                    d2s,
                    psc_sb[:, lc, h : h + 1],
                    pbi_sb[:, lc, h : h + 1],
                    MULT,
                    ADD,
                )
            pt = priorp.tile([128, H, BAND], F32, tag="pt")
            nc.scalar.activation(pt[:], pre[:], EXP)
            for h in range(H):
                nc.sync.dma_start(priorband[h, ts(lc, 128), :], pt[:, h, :])

        emit_prior(0)
        emit_prior(1)

        # q/k projections in transposed layout qT[dm_out, l]; dm chunk 0 first
        # so the first head pair can start as early as possible.
        qT_sb = const.tile([128, 4, L], BF16)
        kT_sb = const.tile([128, 4, L], BF16)
        for m in range(4):
            for src, wsb, dst in ((xq_sb, wq_sb, qT_sb), (xk_sb, wk_sb, kT_sb)):
                for lh in range(2):
                    ps = ps_proj.tile([128, 512], F32, tag="ps")
                    for kc in range(4):
                        nc.tensor.matmul(
                            ps[:],
                            wsb[:, kc, ts(m, 128)],
                            src[:, kc, ts(lh, 512)],
                            start=(kc == 0),
                            stop=(kc == 3),
                        )
                    nc.vector.tensor_copy(dst[:, m, ts(lh, 512)], ps[:])

        # v projection into per-head [s, 64] blocks + ones column
        v_aug = const.tile([128, 8, H, E + 1], BF16)
        nc.vector.memset(v_aug[:, :, :, E], 1.0)
        for sc in range(8):
            ps3 = ps_proj.tile([128, 8, 64], F32, tag="ps")
            for kc in range(4):
                nc.tensor.matmul(
                    ps3[:],
                    xv_sb[:, kc, ts(sc, 128)],
                    wv_sb[:, kc, :],
                    start=(kc == 0),
                    stop=(kc == 3),
                )
            nc.vector.tensor_copy(v_aug[:, sc, :, 0:E], ps3[:])

        # ---- attention: head pairs (rows 0-63 / 64-127 run concurrently) ----
        # remaining prior groups are emitted inside the scores loops so the
        # scheduler can slot their (independent) ACT exps into ScalarE gaps.
        prior_sched = {(0, 2): 2, (0, 5): 3, (1, 2): 4, (1, 5): 5, (2, 2): 6, (3, 2): 7}
        vt_sb = const.tile([128, 4, L], BF16)  # normalized V^T: lhsT of Wo proj
        for c in range(4):
            # scores + exp for heads 2c (rows 0:64) and 2c+1 (rows 64:128)
            expsA = expp.tile([128, 8, L], BF16, tag="exps")
            expsB = expp.tile([128, 8, L], BF16, tag="exps")
            for sc in range(8):
                psA = ps_scores.tile([128, L], F32, tag="sc")
                psB = ps_scores.tile([128, L], F32, tag="sc")
                for lh in range(2):
                    nc.tensor.matmul(
                        psA[:, ts(lh, 512)],
                        kT_sb[0:64, c, ts(sc, 128)],
                        qT_sb[0:64, c, ts(lh, 512)],
                        start=True,
                        stop=True,
                    )
                    nc.tensor.matmul(
                        psB[:, ts(lh, 512)],
                        kT_sb[64:128, c, ts(sc, 128)],
                        qT_sb[64:128, c, ts(lh, 512)],
                        start=True,
                        stop=True,
                    )
                nc.scalar.activation(expsA[:, sc, :], psA[:], EXP, scale=0.125)
                nc.scalar.activation(expsB[:, sc, :], psB[:], EXP, scale=0.125)
                if (c, sc) in prior_sched:
                    emit_prior(prior_sched[(c, sc)])

            # AV: unnormalized heads + softmax sums (ones column, row 64)
            av_t = []
            for i, (h, exps) in enumerate(
                ((2 * c, expsA), (2 * c, expsA), (2 * c + 1, expsB), (2 * c + 1, expsB))
            ):
                lh = i % 2
                pav = ps_av.tile([E + 1, 512], F32)
                for sc in range(8):
                    nc.tensor.matmul(
                        pav[:],
                        v_aug[:, sc, h, :],
                        exps[:, sc, ts(lh, 512)],
                        start=(sc == 0),
                        stop=(sc == 7),
                    )
                av = avp.tile([E, 512], BF16, tag="av")
                nc.vector.tensor_copy(av[:], pav[0:E, :])
                nc.vector.tensor_copy(
                    sums_all[32 * i : 32 * i + 1, :], pav[E : E + 1, :]
                )
                av_t.append(av)

            # batched reciprocal + PE broadcast (selector matmul) + normalize
            rp = smallp.tile([128, 512], F32, tag="rp")
            nc.vector.reciprocal(rp[:], sums_all[:])
            for i in range(4):
                hp, lh = 64 * (i // 2), i % 2
                pb = ps_proj.tile([64, 512], F32, tag="ps")
                nc.tensor.matmul(
                    pb[:], sel_sb[:, ts(i, 64)], rp[:], start=True, stop=True
                )
                nc.vector.tensor_mul(
                    out=vt_sb[hp : hp + 64, c, ts(lh, 512)],
                    in0=av_t[i][:],
                    in1=pb[:],
                )

        # ---- output projection out = V @ Wo ---------------------------------
        for lc in range(8):
            ps = ps_proj.tile([128, 512], F32, tag="ps")
            for cc in range(4):
                nc.tensor.matmul(
                    ps[:],
                    vt_sb[:, cc, ts(lc, 128)],
                    wo_sb[:, cc, :],
                    start=(cc == 0),
                    stop=(cc == 3),
                )
            ot = outsb.tile([128, 512], F32)
            nc.vector.tensor_copy(ot[:], ps[:])
            nc.sync.dma_start(outp[ts(lc, 128), :], ot[:])

    nc.compile()
    return nc


_NC_CACHE = None


def _get_nc():
    global _NC_CACHE
    if _NC_CACHE is None:
        _NC_CACHE = _build()
    return _NC_CACHE


def _install_ntff_hook_shim():
    """The agent image's antenv lacks axon_hooks; recreate it via ctypes
    against libaxon_pjrt.so so run_bass_kernel_spmd(trace=True) can capture
    NTFF profiles (same mechanism as trn_agent_boot)."""
    import contextlib
    import ctypes
    import sys
    import types

    if "antenv.axon_hooks" in sys.modules:
        return
    so_path = "/opt/axon/libaxon_pjrt.so"
    if not os.path.exists(so_path):
        return
    lib = ctypes.CDLL(so_path)
    if not hasattr(lib, "axon_start_nrt_profile"):
        return
    lib.axon_start_nrt_profile.argtypes = [
        ctypes.POINTER(ctypes.c_int64),
        ctypes.c_size_t,
    ]
    lib.axon_start_nrt_profile.restype = ctypes.c_int64
    lib.axon_stop_nrt_profile.argtypes = [ctypes.c_char_p]
    lib.axon_stop_nrt_profile.restype = ctypes.c_int64

    @contextlib.contextmanager
    def _hook(output_dir, device_ids):
        import jax

        jax.devices()
        if device_ids:
            ids = (ctypes.c_int64 * len(device_ids))(*device_ids)
            rc = lib.axon_start_nrt_profile(ids, len(device_ids))
        else:
            rc = lib.axon_start_nrt_profile(None, 0)
        if rc != 0:
            raise RuntimeError(f"axon_start_nrt_profile rc={rc}")
        try:
            yield
        finally:
            n = lib.axon_stop_nrt_profile(str(output_dir).encode())
            print(f"ntff profile: {n} file(s) written to {output_dir}")

    mod = types.ModuleType("antenv.axon_hooks")
    mod.get_axon_ntff_profile_hook = lambda: _hook
    mod.set_axon_ntff_profile_hook = lambda h: None
    sys.modules["antenv.axon_hooks"] = mod


def kernel(queries, keys, values, Wq, Wk, Wv, Wo, Ws):
    global LAST_RESULT
    queries = np.asarray(queries, dtype=np.float32)
    keys = np.asarray(keys, dtype=np.float32)
    values = np.asarray(values, dtype=np.float32)
    Wq = np.asarray(Wq, dtype=np.float32)
    Wk = np.asarray(Wk, dtype=np.float32)
    Wv = np.asarray(Wv, dtype=np.float32)
    Wo = np.asarray(Wo, dtype=np.float32)
    Ws = np.asarray(Ws, dtype=np.float32)

    bf = ml_dtypes.bfloat16

    # sigma -> per-(l,h) scale/bias for the fused prior exp (f32 chain like ref)
    z = (queries.reshape(B * L, DM) @ Ws).reshape(B, L, H).astype(np.float64)
    sig = 1.0 / (1.0 + np.exp(-5.0 * z))
    sigma = np.power(3.0, sig + 1e-5) - 1.0  # [B, L, H], in (0, 2.0002]
    p_scale = (-1.0 / (2.0 * sigma * sigma)).astype(np.float32)
    p_bias = (-0.5 * np.log(2.0 * np.pi) - np.log(sigma)).astype(np.float32)

    # squared-distance table: slice [128-shift : 384-shift] gives (shift+p-j)^2
    p = np.arange(128, dtype=np.float32)[:, None]
    m = np.arange(384, dtype=np.float32)[None, :]
    d2t = (p + 128.0 - m) ** 2

    # selector for the softmax-sum broadcast matmul: sel[32*i, 64*i+j] = 1
    sel = np.zeros((128, 256), dtype=np.float32)
    for i in range(4):
        sel[32 * i, 64 * i : 64 * (i + 1)] = 1.0

    wq_b = np.ascontiguousarray(Wq.astype(bf))
    wk_b = np.ascontiguousarray(Wk.astype(bf))
    wv_b = np.ascontiguousarray(Wv.astype(bf))
    wo_b = np.ascontiguousarray(Wo.astype(bf))

    in_maps = []
    for b in range(B):
        in_maps.append(
            {
                "xqT": np.ascontiguousarray(queries[b].T.astype(bf)),
                "xkT": np.ascontiguousarray(keys[b].T.astype(bf)),
                "xvT": np.ascontiguousarray(values[b].T.astype(bf)),
                "wq": wq_b,
                "wk": wk_b,
                "wv": wv_b,
                "wo": wo_b,
                "d2t": d2t,
                "pscale": np.ascontiguousarray(p_scale[b]),
                "pbias": np.ascontiguousarray(p_bias[b]),
                "sel": sel,
            }
        )

    nc = _get_nc()
    trace = bool(int(os.environ.get("KERNEL_TRACE", "0")))
    if trace:
        try:
            _install_ntff_hook_shim()
        except Exception as e:  # profiling is best-effort
            print(f"ntff hook shim failed: {e}")
    res = run_bass_kernel_spmd(
        nc,
        in_maps,
        core_ids=list(range(8)),
        trace=trace,
    )
    LAST_RESULT = res

    out = np.stack([np.asarray(res.results[b]["outp"]) for b in range(B)])
    prior = np.zeros((B, H, L, L), dtype=np.float32)
    for b in range(B):
        band = np.asarray(res.results[b]["priorband"])  # [H, L, BAND]
        for lc in range(8):
            rows = slice(128 * lc, 128 * lc + 128)
            prior[b, :, rows, W0[lc] : W0[lc] + BAND] = band[:, rows, :]
    return out, prior


# revision 21
# speedup vs baseline: 1.2049x; 1.2049x over previous
"""Anomaly MultiHeadAttention Trainium2 kernel.

Data-parallel over batch B=8 across 8 NeuronCores (one batch element per core,
no collectives). Per core:

  - qT/kT projections produce transposed layouts [dm, l] feeding the scores
    matmul directly; v is produced in natural [s, dm] layout with a ones
    column appended per head so the attention-V matmul also yields softmax
    row sums for free.
  - scoresT[s, l] per head via PE (bf16, fp32 PSUM accumulate); softmax exp on
    ScalarE directly from PSUM with the 1/sqrt(E) scale folded into the
    activation's free affine.  No max subtraction (scores are ~N(0,1)).
  - AV matmul gives out_VT[e,l] + sums row; normalization divides via a
    PE-broadcast reciprocal; result is exactly the lhsT of the final Wo
    projection.
  - prior = 1/(sqrt(2pi)*sigma) * exp(-d^2/(2 sigma^2)) underflows to exactly
    0 in fp32 for |l-s| >= 29 (sigma <= 2.0002), so only a 256-wide band is
    computed (one ScalarE exp per tile: exp(scale_l * d2 + bias_l) with
    per-partition scale/bias precomputed on host) and the host scatters the
    band into the zero-filled full [B,H,L,L] output.

Self-contained: hardcodes shapes; host-side prep = transposes, bf16 casts and
the tiny sigma->scale/bias precompute ([B,L,H], 0.2% of total FLOPs).
"""

import os
from contextlib import ExitStack

import ml_dtypes
import numpy as np

import concourse.bass as bass
import concourse.tile as tile
from concourse import bacc, mybir
from concourse.bass import ts
from concourse.bass_utils import run_bass_kernel_spmd

B, L, DM, H = 8, 1024, 512, 8
E = DM // H  # 64
BAND = 256  # prior band width (covers |l-s| <= 64 minimum for every row)
# window start per 128-row l-chunk (clamped so every chunk reads 256 cols)
W0 = [0, 64, 192, 320, 448, 576, 704, 768]

F32 = mybir.dt.float32
BF16 = mybir.dt.bfloat16

LAST_RESULT = None  # BassKernelResults of the most recent run (for test.py)


def _build():
    nc = bacc.Bacc(
        "TRN2",
        target_bir_lowering=False,
        debug=False,
        enable_asserts=False,
        num_devices=8,
    )

    xqT = nc.dram_tensor("xqT", [DM, L], BF16, kind="ExternalInput").ap()
    xkT = nc.dram_tensor("xkT", [DM, L], BF16, kind="ExternalInput").ap()
    xvT = nc.dram_tensor("xvT", [DM, L], BF16, kind="ExternalInput").ap()
    wq = nc.dram_tensor("wq", [DM, DM], BF16, kind="ExternalInput").ap()
    wk = nc.dram_tensor("wk", [DM, DM], BF16, kind="ExternalInput").ap()
    wv = nc.dram_tensor("wv", [DM, DM], BF16, kind="ExternalInput").ap()
    wo = nc.dram_tensor("wo", [DM, DM], BF16, kind="ExternalInput").ap()
    d2t = nc.dram_tensor("d2t", [128, 384], F32, kind="ExternalInput").ap()
    pscale = nc.dram_tensor("pscale", [L, H], F32, kind="ExternalInput").ap()
    pbias = nc.dram_tensor("pbias", [L, H], F32, kind="ExternalInput").ap()
    sel = nc.dram_tensor("sel", [128, 256], F32, kind="ExternalInput").ap()

    outp = nc.dram_tensor("outp", [L, DM], F32, kind="ExternalOutput").ap()
    priorband = nc.dram_tensor("priorband", [H, L, BAND], F32, kind="ExternalOutput").ap()

    EXP = mybir.ActivationFunctionType.Exp
    MULT = mybir.AluOpType.mult
    ADD = mybir.AluOpType.add

    with tile.TileContext(nc) as tc, ExitStack() as ctx:
        const = ctx.enter_context(tc.tile_pool(name="const", bufs=1))
        priorp = ctx.enter_context(tc.tile_pool(name="priorp", bufs=2))
        expp = ctx.enter_context(tc.tile_pool(name="expp", bufs=4))
        smallp = ctx.enter_context(tc.tile_pool(name="smallp", bufs=3))
        avp = ctx.enter_context(tc.tile_pool(name="avp", bufs=6))
        outsb = ctx.enter_context(tc.tile_pool(name="outsb", bufs=2))
        ps_scores = ctx.enter_context(tc.tile_pool(name="ps_scores", bufs=2, space="PSUM"))
        ps_av = ctx.enter_context(tc.tile_pool(name="ps_av", bufs=2, space="PSUM"))
        ps_proj = ctx.enter_context(tc.tile_pool(name="ps_proj", bufs=2, space="PSUM"))

        # ---- ALL input loads first: the Sync engine issues DMAs in program
        # order, so every load must precede any compute-dependent store (a
        # blocked store would head-of-line-block the loads).
        d2_sb = const.tile([128, 384], F32)
        nc.sync.dma_start(d2_sb[:], d2t)
        psc_sb = const.tile([128, 8, H], F32)
        nc.sync.dma_start(psc_sb[:], pscale.rearrange("(c p) h -> p c h", p=128))
        pbi_sb = const.tile([128, 8, H], F32)
        nc.sync.dma_start(pbi_sb[:], pbias.rearrange("(c p) h -> p c h", p=128))
        sel_sb = const.tile([128, 256], F32)
        nc.sync.dma_start(sel_sb[:], sel)

        wq_sb = const.tile([128, 4, DM], BF16)
        wk_sb = const.tile([128, 4, DM], BF16)
        wv_sb = const.tile([128, 4, DM], BF16)
        wo_sb = const.tile([128, 4, DM], BF16)
        xq_sb = const.tile([128, 4, L], BF16)
        xk_sb = const.tile([128, 4, L], BF16)
        xv_sb = const.tile([128, 4, L], BF16)
        for kc in range(4):
            nc.sync.dma_start(wq_sb[:, kc, :], wq.rearrange("(c p) n -> p c n", p=128)[:, kc, :])
            nc.sync.dma_start(xq_sb[:, kc, :], xqT.rearrange("(c p) l -> p c l", p=128)[:, kc, :])
            nc.sync.dma_start(wk_sb[:, kc, :], wk.rearrange("(c p) n -> p c n", p=128)[:, kc, :])
            nc.sync.dma_start(xk_sb[:, kc, :], xkT.rearrange("(c p) l -> p c l", p=128)[:, kc, :])
            nc.sync.dma_start(wv_sb[:, kc, :], wv.rearrange("(c p) n -> p c n", p=128)[:, kc, :])
            nc.sync.dma_start(xv_sb[:, kc, :], xvT.rearrange("(c p) l -> p c l", p=128)[:, kc, :])
        nc.sync.dma_start(wo_sb[:], wo.rearrange("(c p) n -> p c n", p=128))

        # softmax sums staging: rows live at partitions 0/32/64/96 (32-aligned)
        sums_all = const.tile([128, 512], F32)
        nc.vector.memset(sums_all[:], 1.0)

        # ---- prior band: GPSIMD prescale + one big ACT exp per l_chunk ------
        # prior tile = exp(scale_l * d2 + bias_l); all 8 heads batched in the
        # free dim so ScalarE pays the per-op overhead once per l_chunk.
        def emit_prior(lc):
            shift = 128 * lc - W0[lc]  # 0, 64 or 128
            d2s = d2_sb[:, 128 - shift : 384 - shift]
            pre = priorp.tile([128, H, BAND], F32, tag="pre")
            for h in range(H):
                nc.gpsimd.tensor_scalar(
                    pre[:, h, :],
                    d2s,
                    psc_sb[:, lc, h : h + 1],
                    pbi_sb[:, lc, h : h + 1],
                    MULT,
                    ADD,
                )
            pt = priorp.tile([128, H, BAND], F32, tag="pt")
            nc.scalar.activation(pt[:], pre[:], EXP)
            for h in range(H):
                nc.sync.dma_start(priorband[h, ts(lc, 128), :], pt[:, h, :])

        emit_prior(0)
        emit_prior(1)
        emit_prior(2)

        # q/k projections in transposed layout qT[dm_out, l]; dm chunk 0 first
        # so the first head pair can start as early as possible.
        qT_sb = const.tile([128, 4, L], BF16)
        kT_sb = const.tile([128, 4, L], BF16)
        for m in range(4):
            for src, wsb, dst in ((xq_sb, wq_sb, qT_sb), (xk_sb, wk_sb, kT_sb)):
                for lh in range(2):
                    ps = ps_proj.tile([128, 512], F32, tag="ps")
                    for kc in range(4):
                        nc.tensor.matmul(
                            ps[:],
                            wsb[:, kc, ts(m, 128)],
                            src[:, kc, ts(lh, 512)],
                            start=(kc == 0),
                            stop=(kc == 3),
                        )
                    nc.vector.tensor_copy(dst[:, m, ts(lh, 512)], ps[:])

        # v projection into per-head [s, 64] blocks + ones column
        v_aug = const.tile([128, 8, H, E + 1], BF16)
        nc.vector.memset(v_aug[:, :, :, E], 1.0)
        for sc in range(8):
            ps3 = ps_proj.tile([128, 8, 64], F32, tag="ps")
            for kc in range(4):
                nc.tensor.matmul(
                    ps3[:],
                    xv_sb[:, kc, ts(sc, 128)],
                    wv_sb[:, kc, :],
                    start=(kc == 0),
                    stop=(kc == 3),
                )
            nc.vector.tensor_copy(v_aug[:, sc, :, 0:E], ps3[:])

        # ---- attention: head pairs, software-pipelined ----------------------
        # Pair c heads (2c, 2c+1) map to qT/kT/vt chunk c; rows 0:64 and
        # 64:128 of the PE array run the two heads' scores concurrently
        # (row-group tiling).  The AV matmuls of pair c-1 are interleaved into
        # pair c's scores slots so the PE never drains while ScalarE works
        # through the exps, and ScalarE never waits at a pair boundary.
        vt_sb = const.tile([128, 4, L], BF16)  # normalized V^T: lhsT of Wo proj

        def av_chain_def(p, i, expsA, expsB):
            # chain i of pair p: (head, exps tile, l-half)
            h = 2 * p + (i // 2)
            return h, (expsA if i < 2 else expsB), i % 2

        def emit_av_steps(p, i, pav, expsA, expsB, steps):
            h, exps, lh = av_chain_def(p, i, expsA, expsB)
            for s in steps:
                nc.tensor.matmul(
                    pav[:],
                    v_aug[:, s, h, :],
                    exps[:, s, ts(lh, 512)],
                    start=(s == 0),
                    stop=(s == 7),
                )

        def emit_av_drain(p, i, pav, av_t):
            av = avp.tile([E, 512], BF16, tag="av")
            nc.vector.tensor_copy(av[:], pav[0:E, :])
            nc.vector.tensor_copy(sums_all[32 * i : 32 * i + 1, :], pav[E : E + 1, :])
            av_t.append(av)

        def emit_norm(p, av_t):
            # batched reciprocal + PE broadcast (selector matmul) + normalize
            rp = smallp.tile([128, 512], F32, tag="rp")
            nc.vector.reciprocal(rp[:], sums_all[:])
            for i in range(4):
                hp, lh = 64 * (i // 2), i % 2
                pb = ps_proj.tile([64, 512], F32, tag="ps")
                nc.tensor.matmul(
                    pb[:], sel_sb[:, ts(i, 64)], rp[:], start=True, stop=True
                )
                nc.vector.tensor_mul(
                    out=vt_sb[hp : hp + 64, p, ts(lh, 512)],
                    in0=av_t[i][:],
                    in1=pb[:],
                )

        prev = None  # (pair index, expsA, expsB)
        for c in range(4):
            expsA = expp.tile([128, 8, L], BF16, tag="exps")
            expsB = expp.tile([128, 8, L], BF16, tag="exps")
            av_t = []
            pav0 = pav1 = None
            for sc in range(8):
                psA = ps_scores.tile([128, L], F32, tag="sc")
                psB = ps_scores.tile([128, L], F32, tag="sc")
                for lh in range(2):
                    nc.tensor.matmul(
                        psA[:, ts(lh, 512)],
                        kT_sb[0:64, c, ts(sc, 128)],
                        qT_sb[0:64, c, ts(lh, 512)],
                        start=True,
                        stop=True,
                    )
                    nc.tensor.matmul(
                        psB[:, ts(lh, 512)],
                        kT_sb[64:128, c, ts(sc, 128)],
                        qT_sb[64:128, c, ts(lh, 512)],
                        start=True,
                        stop=True,
                    )
                nc.scalar.activation(expsA[:, sc, :], psA[:], EXP, scale=0.125)
                nc.scalar.activation(expsB[:, sc, :], psB[:], EXP, scale=0.125)

                if prev is not None:
                    p, pA, pB = prev
                    half, step = sc // 4, sc % 4
                    if step == 0:
                        pav0 = ps_av.tile([E + 1, 512], F32, tag="pav")
                        pav1 = ps_av.tile([E + 1, 512], F32, tag="pav")
                    i0, i1 = 2 * half, 2 * half + 1
                    emit_av_steps(p, i0, pav0, pA, pB, (2 * step, 2 * step + 1))
                    emit_av_steps(p, i1, pav1, pA, pB, (2 * step, 2 * step + 1))
                    if step == 3:
                        emit_av_drain(p, i0, pav0, av_t)
                        emit_av_drain(p, i1, pav1, av_t)

            if prev is not None:
                emit_norm(prev[0], av_t)
            prev = (c, expsA, expsB)
            av_t_last = av_t

        # epilogue: AV + norm of the last pair, priors fill ScalarE
        p, pA, pB = prev
        av_t = []
        for half in range(2):
            pav0 = ps_av.tile([E + 1, 512], F32, tag="pav")
            pav1 = ps_av.tile([E + 1, 512], F32, tag="pav")
            emit_av_steps(p, 2 * half, pav0, pA, pB, range(8))
            emit_av_steps(p, 2 * half + 1, pav1, pA, pB, range(8))
            emit_av_drain(p, 2 * half, pav0, av_t)
            emit_av_drain(p, 2 * half + 1, pav1, av_t)
            emit_prior(3 + 2 * half)
            emit_prior(4 + 2 * half)
        emit_prior(7)
        emit_norm(p, av_t)

        # ---- output projection out = V @ Wo ---------------------------------
        for lc in range(8):
            ps = ps_proj.tile([128, 512], F32, tag="ps")
            for cc in range(4):
                nc.tensor.matmul(
                    ps[:],
                    vt_sb[:, cc, ts(lc, 128)],
                    wo_sb[:, cc, :],
                    start=(cc == 0),
                    stop=(cc == 3),
                )
            ot = outsb.tile([128, 512], F32)
            nc.vector.tensor_copy(ot[:], ps[:])
            nc.sync.dma_start(outp[ts(lc, 128), :], ot[:])

    nc.compile()
    return nc


_NC_CACHE = None


def _get_nc():
    global _NC_CACHE
    if _NC_CACHE is None:
        _NC_CACHE = _build()
    return _NC_CACHE


def _install_ntff_hook_shim():
    """The agent image's antenv lacks axon_hooks; recreate it via ctypes
    against libaxon_pjrt.so so run_bass_kernel_spmd(trace=True) can capture
    NTFF profiles (same mechanism as trn_agent_boot)."""
    import contextlib
    import ctypes
    import sys
    import types

    if "antenv.axon_hooks" in sys.modules:
        return
    so_path = "/opt/axon/libaxon_pjrt.so"
    if not os.path.exists(so_path):
        return
    lib = ctypes.CDLL(so_path)
    if not hasattr(lib, "axon_start_nrt_profile"):
        return
    lib.axon_start_nrt_profile.argtypes = [
        ctypes.POINTER(ctypes.c_int64),
        ctypes.c_size_t,
    ]
    lib.axon_start_nrt_profile.restype = ctypes.c_int64
    lib.axon_stop_nrt_profile.argtypes = [ctypes.c_char_p]
    lib.axon_stop_nrt_profile.restype = ctypes.c_int64

    @contextlib.contextmanager
    def _hook(output_dir, device_ids):
        import jax

        jax.devices()
        if device_ids:
            ids = (ctypes.c_int64 * len(device_ids))(*device_ids)
            rc = lib.axon_start_nrt_profile(ids, len(device_ids))
        else:
            rc = lib.axon_start_nrt_profile(None, 0)
        if rc != 0:
            raise RuntimeError(f"axon_start_nrt_profile rc={rc}")
        try:
            yield
        finally:
            n = lib.axon_stop_nrt_profile(str(output_dir).encode())
            print(f"ntff profile: {n} file(s) written to {output_dir}")

    mod = types.ModuleType("antenv.axon_hooks")
    mod.get_axon_ntff_profile_hook = lambda: _hook
    mod.set_axon_ntff_profile_hook = lambda h: None
    sys.modules["antenv.axon_hooks"] = mod


def kernel(queries, keys, values, Wq, Wk, Wv, Wo, Ws):
    global LAST_RESULT
    queries = np.asarray(queries, dtype=np.float32)
    keys = np.asarray(keys, dtype=np.float32)
    values = np.asarray(values, dtype=np.float32)
    Wq = np.asarray(Wq, dtype=np.float32)
    Wk = np.asarray(Wk, dtype=np.float32)
    Wv = np.asarray(Wv, dtype=np.float32)
    Wo = np.asarray(Wo, dtype=np.float32)
    Ws = np.asarray(Ws, dtype=np.float32)

    bf = ml_dtypes.bfloat16

    # sigma -> per-(l,h) scale/bias for the fused prior exp (f32 chain like ref)
    z = (queries.reshape(B * L, DM) @ Ws).reshape(B, L, H).astype(np.float64)
    sig = 1.0 / (1.0 + np.exp(-5.0 * z))
    sigma = np.power(3.0, sig + 1e-5) - 1.0  # [B, L, H], in (0, 2.0002]
    p_scale = (-1.0 / (2.0 * sigma * sigma)).astype(np.float32)
    p_bias = (-0.5 * np.log(2.0 * np.pi) - np.log(sigma)).astype(np.float32)

    # squared-distance table: slice [128-shift : 384-shift] gives (shift+p-j)^2
    p = np.arange(128, dtype=np.float32)[:, None]
    m = np.arange(384, dtype=np.float32)[None, :]
    d2t = (p + 128.0 - m) ** 2

    # selector for the softmax-sum broadcast matmul: sel[32*i, 64*i+j] = 1
    sel = np.zeros((128, 256), dtype=np.float32)
    for i in range(4):
        sel[32 * i, 64 * i : 64 * (i + 1)] = 1.0

    wq_b = np.ascontiguousarray(Wq.astype(bf))
    wk_b = np.ascontiguousarray(Wk.astype(bf))
    wv_b = np.ascontiguousarray(Wv.astype(bf))
    wo_b = np.ascontiguousarray(Wo.astype(bf))

    in_maps = []
    for b in range(B):
        in_maps.append(
            {
                "xqT": np.ascontiguousarray(queries[b].T.astype(bf)),
                "xkT": np.ascontiguousarray(keys[b].T.astype(bf)),
                "xvT": np.ascontiguousarray(values[b].T.astype(bf)),
                "wq": wq_b,
                "wk": wk_b,
                "wv": wv_b,
                "wo": wo_b,
                "d2t": d2t,
                "pscale": np.ascontiguousarray(p_scale[b]),
                "pbias": np.ascontiguousarray(p_bias[b]),
                "sel": sel,
            }
        )

    nc = _get_nc()
    trace = bool(int(os.environ.get("KERNEL_TRACE", "0")))
    if trace:
        try:
            _install_ntff_hook_shim()
        except Exception as e:  # profiling is best-effort
            print(f"ntff hook shim failed: {e}")
    res = run_bass_kernel_spmd(
        nc,
        in_maps,
        core_ids=list(range(8)),
        trace=trace,
    )
    LAST_RESULT = res

    out = np.stack([np.asarray(res.results[b]["outp"]) for b in range(B)])
    prior = np.zeros((B, H, L, L), dtype=np.float32)
    for b in range(B):
        band = np.asarray(res.results[b]["priorband"])  # [H, L, BAND]
        for lc in range(8):
            rows = slice(128 * lc, 128 * lc + 128)
            prior[b, :, rows, W0[lc] : W0[lc] + BAND] = band[:, rows, :]
    return out, prior


# revision 25
# speedup vs baseline: 1.2519x; 1.0390x over previous
"""Anomaly MultiHeadAttention Trainium2 kernel.

Data-parallel over batch B=8 across 8 NeuronCores (one batch element per core,
no collectives). Per core:

  - qT/kT projections produce transposed layouts [dm, l] feeding the scores
    matmul directly; v is produced in natural [s, dm] layout with a ones
    column appended per head so the attention-V matmul also yields softmax
    row sums for free.
  - scoresT[s, l] per head via PE (bf16, fp32 PSUM accumulate); softmax exp on
    ScalarE directly from PSUM with the 1/sqrt(E) scale folded into the
    activation's free affine.  No max subtraction (scores are ~N(0,1)).
  - AV matmul gives out_VT[e,l] + sums row; normalization divides via a
    PE-broadcast reciprocal; result is exactly the lhsT of the final Wo
    projection.
  - prior = 1/(sqrt(2pi)*sigma) * exp(-d^2/(2 sigma^2)) underflows to exactly
    0 in fp32 for |l-s| >= 29 (sigma <= 2.0002), so only a 256-wide band is
    computed (one ScalarE exp per tile: exp(scale_l * d2 + bias_l) with
    per-partition scale/bias precomputed on host) and the host scatters the
    band into the zero-filled full [B,H,L,L] output.

Self-contained: hardcodes shapes; host-side prep = transposes, bf16 casts and
the tiny sigma->scale/bias precompute ([B,L,H], 0.2% of total FLOPs).
"""

import os
from contextlib import ExitStack

import ml_dtypes
import numpy as np

import concourse.bass as bass
import concourse.tile as tile
from concourse import bacc, mybir
from concourse.bass import ts
from concourse.bass_utils import run_bass_kernel_spmd

B, L, DM, H = 8, 1024, 512, 8
E = DM // H  # 64
BAND = 256  # prior band width (covers |l-s| <= 64 minimum for every row)
# window start per 128-row l-chunk (clamped so every chunk reads 256 cols)
W0 = [0, 64, 192, 320, 448, 576, 704, 768]

F32 = mybir.dt.float32
BF16 = mybir.dt.bfloat16

LAST_RESULT = None  # BassKernelResults of the most recent run (for test.py)


def _build():
    nc = bacc.Bacc(
        "TRN2",
        target_bir_lowering=False,
        debug=False,
        enable_asserts=False,
        num_devices=8,
    )

    xqT = nc.dram_tensor("xqT", [DM, L], BF16, kind="ExternalInput").ap()
    xkT = nc.dram_tensor("xkT", [DM, L], BF16, kind="ExternalInput").ap()
    xvT = nc.dram_tensor("xvT", [DM, L], BF16, kind="ExternalInput").ap()
    wq = nc.dram_tensor("wq", [DM, DM], BF16, kind="ExternalInput").ap()
    wk = nc.dram_tensor("wk", [DM, DM], BF16, kind="ExternalInput").ap()
    wv = nc.dram_tensor("wv", [DM, DM], BF16, kind="ExternalInput").ap()
    wo = nc.dram_tensor("wo", [DM, DM], BF16, kind="ExternalInput").ap()
    d2t = nc.dram_tensor("d2t", [128, 384], F32, kind="ExternalInput").ap()
    pscale = nc.dram_tensor("pscale", [L, H], F32, kind="ExternalInput").ap()
    pbias = nc.dram_tensor("pbias", [L, H], F32, kind="ExternalInput").ap()
    sel = nc.dram_tensor("sel", [128, 256], F32, kind="ExternalInput").ap()

    outp = nc.dram_tensor("outp", [L, DM], F32, kind="ExternalOutput").ap()
    priorband = nc.dram_tensor("priorband", [H, L, BAND], F32, kind="ExternalOutput").ap()

    EXP = mybir.ActivationFunctionType.Exp
    MULT = mybir.AluOpType.mult
    ADD = mybir.AluOpType.add

    with tile.TileContext(nc) as tc, ExitStack() as ctx:
        const = ctx.enter_context(tc.tile_pool(name="const", bufs=1))
        priorp = ctx.enter_context(tc.tile_pool(name="priorp", bufs=2))
        expp = ctx.enter_context(tc.tile_pool(name="expp", bufs=4))
        smallp = ctx.enter_context(tc.tile_pool(name="smallp", bufs=3))
        avp = ctx.enter_context(tc.tile_pool(name="avp", bufs=6))
        accp = ctx.enter_context(tc.tile_pool(name="accp", bufs=8))
        outsb = ctx.enter_context(tc.tile_pool(name="outsb", bufs=2))
        ps_scores = ctx.enter_context(tc.tile_pool(name="ps_scores", bufs=2, space="PSUM"))
        ps_av = ctx.enter_context(tc.tile_pool(name="ps_av", bufs=2, space="PSUM"))
        ps_proj = ctx.enter_context(tc.tile_pool(name="ps_proj", bufs=2, space="PSUM"))

        # ---- ALL input loads first: the Sync engine issues DMAs in program
        # order, so every load must precede any compute-dependent store (a
        # blocked store would head-of-line-block the loads).
        d2_sb = const.tile([128, 384], F32)
        nc.sync.dma_start(d2_sb[:], d2t)
        psc_sb = const.tile([128, 8, H], F32)
        nc.sync.dma_start(psc_sb[:], pscale.rearrange("(c p) h -> p c h", p=128))
        pbi_sb = const.tile([128, 8, H], F32)
        nc.sync.dma_start(pbi_sb[:], pbias.rearrange("(c p) h -> p c h", p=128))
        sel_sb = const.tile([128, 256], F32)
        nc.sync.dma_start(sel_sb[:], sel)

        wq_sb = const.tile([128, 4, DM], BF16)
        wk_sb = const.tile([128, 4, DM], BF16)
        wv_sb = const.tile([128, 4, DM], BF16)
        wo_sb = const.tile([128, 4, DM], BF16)
        xq_sb = const.tile([128, 4, L], BF16)
        xk_sb = const.tile([128, 4, L], BF16)
        xv_sb = const.tile([128, 4, L], BF16)
        for kc in range(4):
            nc.sync.dma_start(wq_sb[:, kc, :], wq.rearrange("(c p) n -> p c n", p=128)[:, kc, :])
            nc.sync.dma_start(xq_sb[:, kc, :], xqT.rearrange("(c p) l -> p c l", p=128)[:, kc, :])
            nc.sync.dma_start(wk_sb[:, kc, :], wk.rearrange("(c p) n -> p c n", p=128)[:, kc, :])
            nc.sync.dma_start(xk_sb[:, kc, :], xkT.rearrange("(c p) l -> p c l", p=128)[:, kc, :])
        for kc in range(4):
            nc.sync.dma_start(wv_sb[:, kc, :], wv.rearrange("(c p) n -> p c n", p=128)[:, kc, :])
            nc.sync.dma_start(xv_sb[:, kc, :], xvT.rearrange("(c p) l -> p c l", p=128)[:, kc, :])
        nc.sync.dma_start(wo_sb[:], wo.rearrange("(c p) n -> p c n", p=128))

        # softmax sums staging: rows live at partitions 0/32/64/96 (32-aligned)
        sums_all = const.tile([128, 512], F32)
        nc.vector.memset(sums_all[:], 1.0)

        # ---- PE warmup: dummy matmuls on the d2 table while the input DMAs
        # land, so the HAM clock-gate is at 2.4 GHz when projections start.
        pwu = ps_proj.tile([128, 256], F32, tag="ps")
        for _ in range(12):
            nc.tensor.matmul(pwu[:], d2_sb[:, 0:128], d2_sb[:, 128:384], start=True, stop=True)
        # consumed via sums_all so DCE keeps the warmup chain
        nc.vector.tensor_copy(sums_all[0:1, 0:64], pwu[0:1, 0:64])

        # ---- prior band: GPSIMD prescale + one big ACT exp per l_chunk ------
        # prior tile = exp(scale_l * d2 + bias_l); all 8 heads batched in the
        # free dim so ScalarE pays the per-op overhead once per l_chunk.
        def emit_prior(lc):
            shift = 128 * lc - W0[lc]  # 0, 64 or 128
            d2s = d2_sb[:, 128 - shift : 384 - shift]
            pre = priorp.tile([128, H, BAND], F32, tag="pre")
            for h in range(H):
                nc.gpsimd.tensor_scalar(
                    pre[:, h, :],
                    d2s,
                    psc_sb[:, lc, h : h + 1],
                    pbi_sb[:, lc, h : h + 1],
                    MULT,
                    ADD,
                )
            nc.scalar.activation(pre[:], pre[:], EXP)  # in-place exp
            for h in range(H):
                nc.sync.dma_start(priorband[h, ts(lc, 128), :], pre[:, h, :])

        emit_prior(0)
        emit_prior(1)
        emit_prior(2)

        # q/k projections in transposed layout qT[dm_out, l]; dm chunk 0 first
        # so the first head pair can start as early as possible.
        qT_sb = const.tile([128, 4, L], BF16)
        kT_sb = const.tile([128, 4, L], BF16)
        for m in range(4):
            for src, wsb, dst in ((xq_sb, wq_sb, qT_sb), (xk_sb, wk_sb, kT_sb)):
                for lh in range(2):
                    ps = ps_proj.tile([128, 512], F32, tag="ps")
                    for kc in range(4):
                        nc.tensor.matmul(
                            ps[:],
                            wsb[:, kc, ts(m, 128)],
                            src[:, kc, ts(lh, 512)],
                            start=(kc == 0),
                            stop=(kc == 3),
                        )
                    nc.vector.tensor_copy(dst[:, m, ts(lh, 512)], ps[:])

        # v projection into per-head [s, 64] blocks + ones column
        v_aug = const.tile([128, 8, H, E + 1], BF16)
        nc.vector.memset(v_aug[:, :, :, E], 1.0)
        for sc in range(8):
            ps3 = ps_proj.tile([128, 8, 64], F32, tag="ps")
            for kc in range(4):
                nc.tensor.matmul(
                    ps3[:],
                    xv_sb[:, kc, ts(sc, 128)],
                    wv_sb[:, kc, :],
                    start=(kc == 0),
                    stop=(kc == 3),
                )
            nc.vector.tensor_copy(v_aug[:, sc, :, 0:E], ps3[:])

        # ---- attention: head pairs, software-pipelined ----------------------
        # Pair c heads (2c, 2c+1) map to qT/kT/vt chunk c; rows 0:64 and
        # 64:128 of the PE array run the two heads' scores concurrently
        # (row-group tiling).  The AV matmuls of pair c-1 are interleaved into
        # pair c's scores slots so the PE never drains while ScalarE works
        # through the exps, and ScalarE never waits at a pair boundary.
        vt_sb = const.tile([128, 4, L], BF16)  # normalized V^T: lhsT of Wo proj

        def av_chain_def(p, i, expsA, expsB):
            # chain i of pair p: (head, exps tile, l-half)
            h = 2 * p + (i // 2)
            return h, (expsA if i < 2 else expsB), i % 2

        def emit_av_steps(p, i, pav, expsA, expsB, steps):
            h, exps, lh = av_chain_def(p, i, expsA, expsB)
            for s in steps:
                nc.tensor.matmul(
                    pav[:],
                    v_aug[:, s, h, :],
                    exps[:, s, ts(lh, 512)],
                    start=(s == 0),
                    stop=(s == 7),
                )

        def emit_av_drain(p, i, pav, av_t):
            av = avp.tile([E, 512], BF16, tag="av")
            nc.vector.tensor_copy(av[:], pav[0:E, :])
            nc.vector.tensor_copy(sums_all[32 * i : 32 * i + 1, :], pav[E : E + 1, :])
            av_t.append(av)

        def emit_norm(p, av_t):
            # batched reciprocal + PE broadcast (selector matmul) + normalize
            rp = smallp.tile([128, 512], F32, tag="rp")
            nc.vector.reciprocal(rp[:], sums_all[:])
            for i in range(4):
                hp, lh = 64 * (i // 2), i % 2
                pb = ps_proj.tile([64, 512], F32, tag="ps")
                nc.tensor.matmul(
                    pb[:], sel_sb[:, ts(i, 64)], rp[:], start=True, stop=True
                )
                nc.vector.tensor_mul(
                    out=vt_sb[hp : hp + 64, p, ts(lh, 512)],
                    in0=av_t[i][:],
                    in1=pb[:],
                )

        prev = None  # (pair index, expsA, expsB)
        for c in range(4):
            expsA = expp.tile([128, 8, L], BF16, tag="exps")
            expsB = expp.tile([128, 8, L], BF16, tag="exps")
            av_t = []
            pav0 = pav1 = None
            for sc in range(8):
                psA = ps_scores.tile([128, L], F32, tag="sc")
                psB = ps_scores.tile([128, L], F32, tag="sc")
                for lh in range(2):
                    nc.tensor.matmul(
                        psA[:, ts(lh, 512)],
                        kT_sb[0:64, c, ts(sc, 128)],
                        qT_sb[0:64, c, ts(lh, 512)],
                        start=True,
                        stop=True,
                    )
                    nc.tensor.matmul(
                        psB[:, ts(lh, 512)],
                        kT_sb[64:128, c, ts(sc, 128)],
                        qT_sb[64:128, c, ts(lh, 512)],
                        start=True,
                        stop=True,
                    )
                nc.scalar.activation(expsA[:, sc, :], psA[:], EXP, scale=0.125)
                nc.scalar.activation(expsB[:, sc, :], psB[:], EXP, scale=0.125)

                if prev is not None:
                    p, pA, pB = prev
                    half, step = sc // 4, sc % 4
                    if step == 0:
                        pav0 = ps_av.tile([E + 1, 512], F32, tag="pav")
                        pav1 = ps_av.tile([E + 1, 512], F32, tag="pav")
                    i0, i1 = 2 * half, 2 * half + 1
                    emit_av_steps(p, i0, pav0, pA, pB, (2 * step, 2 * step + 1))
                    emit_av_steps(p, i1, pav1, pA, pB, (2 * step, 2 * step + 1))
                    if step == 3:
                        emit_av_drain(p, i0, pav0, av_t)
                        emit_av_drain(p, i1, pav1, av_t)

            if prev is not None:
                emit_norm(prev[0], av_t)
            prev = (c, expsA, expsB)
            av_t_last = av_t

        # epilogue: output-projection chunks 0-2 (vt chunks already final) and
        # the last pair's AV run together on the PE; priors fill ScalarE.
        p, pA, pB = prev
        av_t = []
        acc = {}

        def emit_proj_partial(lc):
            ps = ps_proj.tile([128, 512], F32, tag="ps")
            for cc in range(3):
                nc.tensor.matmul(
                    ps[:],
                    vt_sb[:, cc, ts(lc, 128)],
                    wo_sb[:, cc, :],
                    start=(cc == 0),
                    stop=(cc == 2),
                )
            a = accp.tile([128, 512], BF16, tag="acc")
            nc.vector.tensor_copy(a[:], ps[:])
            acc[lc] = a

        for half in range(2):
            pav0 = ps_av.tile([E + 1, 512], F32, tag="pav")
            pav1 = ps_av.tile([E + 1, 512], F32, tag="pav")
            emit_av_steps(p, 2 * half, pav0, pA, pB, range(8))
            emit_av_steps(p, 2 * half + 1, pav1, pA, pB, range(8))
            for lc in range(4 * half, 4 * half + 4):
                emit_proj_partial(lc)
            emit_av_drain(p, 2 * half, pav0, av_t)
            emit_av_drain(p, 2 * half + 1, pav1, av_t)
            emit_prior(3 + 2 * half)
            emit_prior(4 + 2 * half)
        emit_prior(7)
        emit_norm(p, av_t)

        # final: last Wo chunk (needs vt chunk 3) + accumulated partials
        for lc in range(8):
            ps = ps_proj.tile([128, 512], F32, tag="ps")
            nc.tensor.matmul(
                ps[:], vt_sb[:, 3, ts(lc, 128)], wo_sb[:, 3, :], start=True, stop=True
            )
            ot = outsb.tile([128, 512], F32)
            nc.vector.tensor_add(out=ot[:], in0=ps[:], in1=acc[lc][:])
            nc.sync.dma_start(outp[ts(lc, 128), :], ot[:])

    nc.compile()
    return nc


_NC_CACHE = None


def _get_nc():
    global _NC_CACHE
    if _NC_CACHE is None:
        _NC_CACHE = _build()
    return _NC_CACHE


def _install_ntff_hook_shim():
    """The agent image's antenv lacks axon_hooks; recreate it via ctypes
    against libaxon_pjrt.so so run_bass_kernel_spmd(trace=True) can capture
    NTFF profiles (same mechanism as trn_agent_boot)."""
    import contextlib
    import ctypes
    import sys
    import types

    if "antenv.axon_hooks" in sys.modules:
        return
    so_path = "/opt/axon/libaxon_pjrt.so"
    if not os.path.exists(so_path):
        return
    lib = ctypes.CDLL(so_path)
    if not hasattr(lib, "axon_start_nrt_profile"):
        return
    lib.axon_start_nrt_profile.argtypes = [
        ctypes.POINTER(ctypes.c_int64),
        ctypes.c_size_t,
    ]
    lib.axon_start_nrt_profile.restype = ctypes.c_int64
    lib.axon_stop_nrt_profile.argtypes = [ctypes.c_char_p]
    lib.axon_stop_nrt_profile.restype = ctypes.c_int64

    @contextlib.contextmanager
    def _hook(output_dir, device_ids):
        import jax

        jax.devices()
        if device_ids:
            ids = (ctypes.c_int64 * len(device_ids))(*device_ids)
            rc = lib.axon_start_nrt_profile(ids, len(device_ids))
        else:
            rc = lib.axon_start_nrt_profile(None, 0)
        if rc != 0:
            raise RuntimeError(f"axon_start_nrt_profile rc={rc}")
        try:
            yield
        finally:
            n = lib.axon_stop_nrt_profile(str(output_dir).encode())
            print(f"ntff profile: {n} file(s) written to {output_dir}")

    mod = types.ModuleType("antenv.axon_hooks")
    mod.get_axon_ntff_profile_hook = lambda: _hook
    mod.set_axon_ntff_profile_hook = lambda h: None
    sys.modules["antenv.axon_hooks"] = mod


def kernel(queries, keys, values, Wq, Wk, Wv, Wo, Ws):
    global LAST_RESULT
    queries = np.asarray(queries, dtype=np.float32)
    keys = np.asarray(keys, dtype=np.float32)
    values = np.asarray(values, dtype=np.float32)
    Wq = np.asarray(Wq, dtype=np.float32)
    Wk = np.asarray(Wk, dtype=np.float32)
    Wv = np.asarray(Wv, dtype=np.float32)
    Wo = np.asarray(Wo, dtype=np.float32)
    Ws = np.asarray(Ws, dtype=np.float32)

    bf = ml_dtypes.bfloat16

    # sigma -> per-(l,h) scale/bias for the fused prior exp (f32 chain like ref)
    z = (queries.reshape(B * L, DM) @ Ws).reshape(B, L, H).astype(np.float64)
    sig = 1.0 / (1.0 + np.exp(-5.0 * z))
    sigma = np.power(3.0, sig + 1e-5) - 1.0  # [B, L, H], in (0, 2.0002]
    p_scale = (-1.0 / (2.0 * sigma * sigma)).astype(np.float32)
    p_bias = (-0.5 * np.log(2.0 * np.pi) - np.log(sigma)).astype(np.float32)

    # squared-distance table: slice [128-shift : 384-shift] gives (shift+p-j)^2
    p = np.arange(128, dtype=np.float32)[:, None]
    m = np.arange(384, dtype=np.float32)[None, :]
    d2t = (p + 128.0 - m) ** 2

    # selector for the softmax-sum broadcast matmul: sel[32*i, 64*i+j] = 1
    sel = np.zeros((128, 256), dtype=np.float32)
    for i in range(4):
        sel[32 * i, 64 * i : 64 * (i + 1)] = 1.0

    wq_b = np.ascontiguousarray(Wq.astype(bf))
    wk_b = np.ascontiguousarray(Wk.astype(bf))
    wv_b = np.ascontiguousarray(Wv.astype(bf))
    wo_b = np.ascontiguousarray(Wo.astype(bf))

    in_maps = []
    for b in range(B):
        in_maps.append(
            {
                "xqT": np.ascontiguousarray(queries[b].T.astype(bf)),
                "xkT": np.ascontiguousarray(keys[b].T.astype(bf)),
                "xvT": np.ascontiguousarray(values[b].T.astype(bf)),
                "wq": wq_b,
                "wk": wk_b,
                "wv": wv_b,
                "wo": wo_b,
                "d2t": d2t,
                "pscale": np.ascontiguousarray(p_scale[b]),
                "pbias": np.ascontiguousarray(p_bias[b]),
                "sel": sel,
            }
        )

    nc = _get_nc()
    trace = bool(int(os.environ.get("KERNEL_TRACE", "0")))
    if trace:
        try:
            _install_ntff_hook_shim()
        except Exception as e:  # profiling is best-effort
            print(f"ntff hook shim failed: {e}")
    res = run_bass_kernel_spmd(
        nc,
        in_maps,
        core_ids=list(range(8)),
        trace=trace,
    )
    LAST_RESULT = res

    out = np.stack([np.asarray(res.results[b]["outp"]) for b in range(B)])
    prior = np.zeros((B, H, L, L), dtype=np.float32)
    for b in range(B):
        band = np.asarray(res.results[b]["priorband"])  # [H, L, BAND]
        for lc in range(8):
            rows = slice(128 * lc, 128 * lc + 128)
            prior[b, :, rows, W0[lc] : W0[lc] + BAND] = band[:, rows, :]
    return out, prior
